# revision 6
# baseline (speedup 1.0000x reference)
"""GATv2 message passing on 8 Trainium2 NeuronCores (Bass/Tile).

Sharding: nodes are split into 8 contiguous ranges (one per core); every
edge is owned by its receiver's core, so each core computes the complete
softmax + weighted aggregation for its own nodes with no collectives.

Host-side preprocessing (index-driven data movement only):
  * nodes are LPT-packed into (core, tile) bins by in-degree so every
    tile's edge count is balanced; edges are binned by their receiver's
    bin, padded to a common chunk count (T_max chunks of 128 -- the
    balancing makes T_max the minimum possible, 16 for this graph);
  * raw sender node features are pre-gathered per edge (nodes[senders])
    so the device reads sequential streams;
  * both big streams (edge feats, gathered sender feats) and the node
    features / weights are bf16 -> half the HBM traffic of fp32 and
    full-rate matmuls with fast weight loads. The softmax / accumulation
    arithmetic stays fp32 in PSUM. Measured end-to-end normalized error
    vs the fp32 reference is ~5e-3 (gate: 2e-2).

Device pipeline per receiver tile (128 nodes), per group of <=4 edge
chunks (chunk = 128 edges); "T" suffix = transposed [feat, edge] layout:
  bc   = ones.T @ recv_row          broadcast receiver ids   (PE K=1)
  ohne = (bc == iota_p)             one-hot [node, edge]     (DVE)
  zT   = We.T@edgT + Ws.T@srtT + hr_tile@ohne   (PE, PSUM accumulate)
  x    = PRelu(zT, 0.01)                        (ACT)
  lgT  = ablk.T @ x                 logits [8, W]            (PE)
  exT  = Exp(lgT)                                            (ACT)
  exP  = exT.T per chunk            (PE transpose via I8)
  spj  = srtT_chunk.T @ Ws          sender proj [edge, feat] (PE)
  rhs  = [spj * bcast(exP) | exP]   [edge, 136]  (ACT copy + DVE mult)
  ohen = (iota == recv_slot)        one-hot [edge, node]     (DVE)
  acc += ohen.T @ rhs               scatter matmul, accumulates over the
                                    tile's chunks (PSUM)
Epilogue per tile: out = acc[:, :128] / (acc[:, 128:136] + eps) -> DRAM.

Edge-feature streams are fetched 3 tiles per DMA (~1.8 MB) to stay on
the high-efficiency side of the DMA size curve.

Benchmarking (GAT_BENCH=1): the axon tunnel adds ~70-90 ms latency per
dispatch but pipelines back-to-back dispatches, so wall-clock of a
single call measures the network, not the kernel.  HW exec time is
measured as the slope of total wall time vs number of queued dispatches
of a program whose body repeats GAT_REPEAT times -> steady-state
per-iteration device time with the fixed dispatch latency cancelled.
"""
import os
import sys

sys.path.insert(0, "/opt/trn_rl_repo")

import numpy as np
import ml_dtypes

import concourse.bass as bass
import concourse.bacc as bacc
import concourse.mybir as mybir
import concourse.tile as tile

F32 = mybir.dt.float32
BF16 = mybir.dt.bfloat16
BF_NP = ml_dtypes.bfloat16

NCORES = 8
P = 128
HEADS = 8
HDIM = 16

LAST_EXEC_NS = None
LAST_BENCH_NS = None


# --------------------------------------------------------------------------
# host-side sharding / layout
# --------------------------------------------------------------------------
def _prep(nodes, edges, senders, receivers, n_cores):
    import heapq

    N, D = nodes.shape
    E = edges.shape[0]
    assert D == 128
    NLOC = -(-N // n_cores)
    NTILES = (NLOC + P - 1) // P
    NLOC_PAD = NTILES * P
    NBINS = n_cores * NTILES

    # --- LPT node->tile binning: nodes may go to ANY (core, tile) slot
    # (the host unshards by permutation), so balance per-tile in-degree
    # sums.  With balanced bins every tile needs the same, minimal chunk
    # count instead of the global worst case. ---
    deg = np.bincount(receivers, minlength=N)
    order_nodes = np.argsort(-deg, kind="stable")
    bin_of = np.empty(N, dtype=np.int64)
    slot_of = np.empty(N, dtype=np.int64)
    fill = np.zeros(NBINS, dtype=np.int64)
    heap = [(0, b) for b in range(NBINS)]
    heapq.heapify(heap)
    spill = []
    for v in order_nodes:
        while True:
            s, b = heapq.heappop(heap)
            if fill[b] < P:
                break
        bin_of[v] = b
        slot_of[v] = fill[b]
        fill[b] += 1
        if fill[b] < P:
            heapq.heappush(heap, (s + int(deg[v]), b))
        else:
            spill.append((s + int(deg[v]), b))

    gt = bin_of[receivers]
    recv_local = slot_of[receivers].astype(np.float32)

    order = np.argsort(gt, kind="stable")
    gt_sorted = gt[order]
    cnt = np.bincount(gt_sorted, minlength=NBINS)
    T_max = max(1, int(-(-cnt.max() // P)))
    NCHUNK = NTILES * T_max
    E_pad = NCHUNK * P

    starts = np.zeros(NBINS + 1, dtype=np.int64)
    np.cumsum(cnt, out=starts[1:])
    rank = np.arange(E, dtype=np.int64) - starts[gt_sorted]
    slot = (gt_sorted % NTILES) * (T_max * P) + rank

    nodes_bf = nodes.astype(BF_NP)
    edges_bf = edges.astype(BF_NP)
    sent_bf = nodes_bf[senders]
    rl_sorted = recv_local[order]

    EDG = np.zeros((n_cores, P, E_pad), dtype=BF_NP)
    SRT = np.zeros((n_cores, P, E_pad), dtype=BF_NP)
    RROW = np.full((n_cores, 1, E_pad), -1.0, dtype=BF_NP)
    RLOC = np.full((n_cores, P, NCHUNK), -1.0, dtype=np.float32)
    for ci in range(n_cores):
        m = gt_sorted // NTILES == ci
        sel = order[m]
        sl = slot[m]
        EDG[ci][:, sl] = edges_bf[sel].T
        SRT[ci][:, sl] = sent_bf[sel].T
        RROW[ci][0, sl] = rl_sorted[m]
        RLOC[ci][sl % P, sl // P] = rl_sorted[m]

    # node features laid out in permuted (bin, slot) order per core
    NLT = np.zeros((n_cores, P, NLOC_PAD), dtype=BF_NP)
    pos = bin_of * P + slot_of              # global padded position
    core_of = bin_of // NTILES
    loc = pos - core_of * NLOC_PAD
    for ci in range(n_cores):
        mv = core_of == np.int64(ci)
        NLT[ci][:, loc[mv]] = nodes_bf[mv].T

    return dict(EDG=EDG, SRT=SRT, RROW=RROW, RLOC=RLOC, NLT=NLT,
                NLOC=NLOC, NTILES=NTILES, NLOC_PAD=NLOC_PAD,
                T_max=T_max, NCHUNK=NCHUNK, E_pad=E_pad,
                pos_of_node=pos)


def _const_block(Ws_k, Wr_k, We_k, a):
    """bf16 const block (fp32 constants stored bitcast, 2 cols per fp32):
      0:128 We | 128:256 Ws | 256:384 Wr | 384:392 ablk | 392:400 id8
      400:656 iota rows fp32 (iota[p, j] = j) | 656:658 iotaC fp32 (= p)
      658:786 ones
    """
    ablk = np.zeros((P, HEADS), dtype=np.float32)
    for h in range(HEADS):
        ablk[h * HDIM:(h + 1) * HDIM, h] = a[h]
    CW = 786
    C = np.zeros((P, CW), dtype=BF_NP)
    C[:, 0:128] = We_k.astype(BF_NP)
    C[:, 128:256] = Ws_k.astype(BF_NP)
    C[:, 256:384] = Wr_k.astype(BF_NP)
    C[:, 384:392] = ablk.astype(BF_NP)
    C[0:8, 392:400] = np.eye(8, dtype=np.float32).astype(BF_NP)
    C[:, 658:786] = np.ones((P, 128), np.float32).astype(BF_NP)
    Cu = C.view(np.uint16)
    iotaF = np.tile(np.arange(128, dtype=np.float32), (P, 1))
    Cu[:, 400:656] = iotaF.view(np.uint16)
    iotaC = np.arange(P, dtype=np.float32).reshape(P, 1)
    Cu[:, 656:658] = iotaC.view(np.uint16)
    return C, CW


# --------------------------------------------------------------------------
# device program
# --------------------------------------------------------------------------
def _build(NTILES, T_max, NCHUNK, E_pad, NLOC_PAD, CW, group_chunks=4,
           repeat=1, dma_tiles=3, sbg_bufs=3):
    PRELU = mybir.ActivationFunctionType.Prelu
    EXP = mybir.ActivationFunctionType.Exp
    COPY = mybir.ActivationFunctionType.Copy
    EQ = mybir.AluOpType.is_equal
    MUL = mybir.AluOpType.mult
    ADD = mybir.AluOpType.add

    GROUPS = []
    g0 = 0
    while g0 < T_max:
        GROUPS.append((g0, min(group_chunks, T_max - g0)))
        g0 += group_chunks

    nc = bacc.Bacc("TRN2", target_bir_lowering=False, debug=False)

    d_edg = nc.declare_dram_parameter("EDG", [P, E_pad], BF16, isOutput=False)
    d_srt = nc.declare_dram_parameter("SRT", [P, E_pad], BF16, isOutput=False)
    d_rrow = nc.declare_dram_parameter("RROW", [1, E_pad], BF16, isOutput=False)
    d_rloc = nc.declare_dram_parameter("RLOC", [P, NCHUNK], F32, isOutput=False)
    d_nlt = nc.declare_dram_parameter("NLT", [P, NLOC_PAD], BF16, isOutput=False)
    d_cb = nc.declare_dram_parameter("CONST", [P, CW], BF16, isOutput=False)
    d_out = nc.declare_dram_parameter("OUT", [NLOC_PAD, P], F32, isOutput=True)

    with tile.TileContext(nc) as tc:
        with (
            tc.tile_pool(name="cst", bufs=1) as cpool,
            tc.tile_pool(name="sb", bufs=2) as sb,
            tc.tile_pool(name="sbg", bufs=sbg_bufs) as sbg,
            tc.tile_pool(name="ps_z", bufs=2, space="PSUM") as ps_z,
            tc.tile_pool(name="ps_spj", bufs=2, space="PSUM") as ps_spj,
            tc.tile_pool(name="ps_bc", bufs=1, space="PSUM") as ps_bc,
            tc.tile_pool(name="ps_aux", bufs=1, space="PSUM") as ps_aux,
            tc.tile_pool(name="ps_ex", bufs=1, space="PSUM") as ps_ex,
            tc.tile_pool(name="ps_acc", bufs=1, space="PSUM") as ps_acc,
        ):
            cb = cpool.tile([P, CW], BF16)
            nc.sync.dma_start(out=cb[:], in_=d_cb[:])
            rloc = cpool.tile([P, NCHUNK], F32)
            nc.sync.dma_start(out=rloc[:], in_=d_rloc[:])
            nlt = cpool.tile([P, NLOC_PAD], BF16)
            nc.sync.dma_start(out=nlt[:], in_=d_nlt[:])
            hr_sb = cpool.tile([P, NLOC_PAD], BF16)

            c_We = cb[:, 0:128]
            c_Ws = cb[:, 128:256]
            c_Wr = cb[:, 256:384]
            c_ablk = cb[:, 384:392]
            c_id8 = cb[0:8, 392:400]
            c_iota = cb[:, 400:656].bitcast(F32)
            c_iotaC = cb[:, 656:658].bitcast(F32)
            c_ones1 = cb[0:1, 658:786]

            for _rep in range(repeat):
                # ---- prologue: hr projection for the local nodes ----
                for t in range(NTILES):
                    ppz = ps_z.tile([P, group_chunks * P], F32, tag="zT")
                    pp = ppz[:, 0:128]
                    nc.tensor.matmul(
                        out=pp, lhsT=nlt[:, t * P:(t + 1) * P], rhs=c_Wr,
                        start=True, stop=True,
                    )
                    dst = hr_sb[:, t * P:(t + 1) * P]
                    if t % 2 == 0:
                        nc.scalar.activation(dst, pp, COPY)
                    else:
                        nc.vector.tensor_copy(out=dst, in_=pp)

                # ---- main loop over receiver tiles ----
                for t in range(NTILES):
                    if t % dma_tiles == 0:
                        nt = min(dma_tiles, NTILES - t)
                        co = t * T_max * P
                        sz = nt * T_max * P
                        edg_b = sb.tile([P, dma_tiles * T_max * P], BF16,
                                        tag="edg")
                        nc.sync.dma_start(out=edg_b[:, 0:sz],
                                          in_=d_edg[:, co:co + sz])
                        srt_b = sb.tile([P, dma_tiles * T_max * P], BF16,
                                        tag="srt")
                        nc.sync.dma_start(out=srt_b[:, 0:sz],
                                          in_=d_srt[:, co:co + sz])
                        rr_b = sb.tile([1, dma_tiles * T_max * P], BF16,
                                       tag="rr")
                        nc.sync.dma_start(out=rr_b[:, 0:sz],
                                          in_=d_rrow[:, co:co + sz])
                    toff = (t % dma_tiles) * T_max * P
                    edg = edg_b[:, toff:toff + T_max * P]
                    srt = srt_b[:, toff:toff + T_max * P]
                    rr = rr_b[:, toff:toff + T_max * P]

                    acc = ps_acc.tile([P, 136], F32, tag="acc")
                    hr_t = hr_sb[:, t * P:(t + 1) * P]
                    n_sc = 0

                    for gi, (gc0, ncg) in enumerate(GROUPS):
                        W = ncg * P
                        csl = slice(gc0 * P, gc0 * P + W)

                        # receiver-id broadcast -> one-hot [node, edge]
                        ohne = sbg.tile([P, W], BF16, tag="ohne")
                        bc = ps_bc.tile([P, W], F32, tag="bc")
                        nc.tensor.matmul(out=bc[:], lhsT=c_ones1,
                                         rhs=rr[0:1, csl],
                                         start=True, stop=True)
                        nc.vector.tensor_scalar(
                            out=ohne[:], in0=bc[:],
                            scalar1=c_iotaC, scalar2=None, op0=EQ)

                        zT = ps_z.tile([P, group_chunks * P], F32, tag="zT")
                        zT = zT[:, 0:W]
                        nc.tensor.matmul(out=zT, lhsT=c_We, rhs=edg[:, csl],
                                         start=True, stop=False)
                        nc.tensor.matmul(out=zT, lhsT=c_Ws, rhs=srt[:, csl],
                                         start=False, stop=False)
                        nc.tensor.matmul(out=zT, lhsT=hr_t, rhs=ohne[:],
                                         start=False, stop=True)

                        x = sbg.tile([P, W], BF16, tag="x")
                        nc.scalar.activation(x[:], zT, PRELU, alpha=0.01)

                        lg = ps_aux.tile([8, W], F32, tag="lg")
                        nc.tensor.matmul(out=lg[:], lhsT=c_ablk, rhs=x[:],
                                         start=True, stop=True)
                        exT = sbg.tile([8, W], BF16, tag="exT")
                        nc.scalar.activation(exT[:], lg[:], EXP)

                        exP = ps_ex.tile([P, ncg * 8], F32, tag="exP")
                        for c in range(ncg):
                            nc.tensor.matmul(
                                out=exP[:, c * 8:(c + 1) * 8],
                                lhsT=exT[:, c * P:(c + 1) * P], rhs=c_id8,
                                start=True, stop=True,
                            )

                        spj = ps_spj.tile([P, ncg, 128], F32, tag="spj")
                        for c in range(ncg):
                            nc.tensor.matmul(
                                out=spj[:, c, :],
                                lhsT=srt[:, (gc0 + c) * P:(gc0 + c + 1) * P],
                                rhs=c_Ws, start=True, stop=True,
                            )

                        rhs136 = sbg.tile([P, ncg, 136], BF16, tag="rhs136")
                        nc.scalar.activation(
                            rhs136[:, :, 128:136],
                            exP[:].rearrange("p (c h) -> p c h", c=ncg),
                            COPY,
                        )
                        nc.vector.tensor_tensor(
                            out=rhs136[:, :, 0:128],
                            in0=spj[:],
                            in1=rhs136[:, :, 128:136].to_broadcast(
                                [P, ncg, 8, 16]),
                            op=MUL,
                        )

                        # one-hot [edge, node] for the scatter
                        ohen = sbg.tile([P, ncg, 128], BF16, tag="ohen")
                        _rl = rloc[:, t * T_max + gc0: t * T_max + gc0 + ncg]
                        _rlb = bass.AP(_rl.tensor, _rl.offset,
                                       [_rl.ap[0], _rl.ap[1], [0, 128]])
                        _iob = bass.AP(c_iota.tensor, c_iota.offset,
                                       [c_iota.ap[0], [0, ncg], c_iota.ap[1]])
                        nc.vector.tensor_tensor(out=ohen[:], in0=_iob,
                                                in1=_rlb, op=EQ)

                        for c in range(ncg):
                            n_sc += 1
                            nc.tensor.matmul(
                                out=acc[:],
                                lhsT=ohen[:, c, :],
                                rhs=rhs136[:, c, :],
                                start=(n_sc == 1), stop=(n_sc == T_max),
                            )

                    # ---- epilogue: divide by softmax denominators ----
                    dsb = sb.tile([P, 8], F32, tag="dsb")
                    nc.vector.tensor_scalar(out=dsb[:], in0=acc[:, 128:136],
                                            scalar1=1e-30, scalar2=None,
                                            op0=ADD)
                    rec = sb.tile([P, 8], F32, tag="rec")
                    nc.vector.reciprocal(out=rec[:], in_=dsb[:])
                    ot = sb.tile([P, P], F32, tag="ot")
                    nc.vector.tensor_tensor(
                        out=ot[:].rearrange("p (h j) -> p h j", h=8),
                        in0=acc[:, 0:128].rearrange("p (h j) -> p h j", h=8),
                        in1=rec[:].to_broadcast([P, 8, 16]),
                        op=MUL,
                    )
                    nc.sync.dma_start(out=d_out[t * P:(t + 1) * P, :],
                                      in_=ot[:])

    nc.compile()
    return nc


# --------------------------------------------------------------------------
# execution via PJRT on the axon-tunneled cores
# --------------------------------------------------------------------------
def _make_fn(nc, in_maps, n_cores):
    import jax
    from jax.sharding import Mesh, PartitionSpec, NamedSharding
    from jax.experimental.shard_map import shard_map
    import concourse.mybir as _mb
    from concourse import bass2jax as _b2j

    _b2j.install_neuronx_cc_hook()

    in_names, out_names, out_avals, zero_outs = [], [], [], []
    for alloc in nc.m.functions[0].allocations:
        if not isinstance(alloc, _mb.MemoryLocationSet):
            continue
        name = alloc.memorylocations[0].name
        if alloc.kind == "ExternalInput":
            if nc.partition_id_tensor is None or name != nc.partition_id_tensor.name:
                in_names.append(name)
        elif alloc.kind == "ExternalOutput":
            out_names.append(name)
            shape = tuple(alloc.tensor_shape)
            dtype = _mb.dt.np(alloc.dtype)
            out_avals.append(jax.core.ShapedArray(shape, dtype))
            zero_outs.append(np.zeros(shape, dtype))
    n_params = len(in_names)
    in_names_all = in_names + out_names
    part_name = nc.partition_id_tensor.name if nc.partition_id_tensor else None
    if part_name is not None:
        in_names_all.append(part_name)

    def _body(*args):
        operands = list(args)
        if part_name is not None:
            operands.append(_b2j.partition_id_tensor())
        outs = _b2j._bass_exec_p.bind(
            *operands,
            out_avals=tuple(out_avals),
            in_names=tuple(in_names_all),
            out_names=tuple(out_names),
            lowering_input_output_aliases=(),
            sim_require_finite=True,
            sim_require_nnan=True,
            nc=nc,
        )
        return tuple(outs)

    devices = jax.devices()[:n_cores]
    mesh = Mesh(np.asarray(devices), ("core",))
    n_outs = len(out_avals)
    in_specs = (PartitionSpec("core"),) * (n_params + n_outs)
    out_specs = (PartitionSpec("core"),) * n_outs
    fn = jax.jit(
        shard_map(_body, mesh=mesh, in_specs=in_specs,
                  out_specs=out_specs, check_rep=False),
        keep_unused=True,
    )
    sh = NamedSharding(mesh, PartitionSpec("core"))
    concat_in = [
        jax.device_put(
            np.concatenate([np.asarray(in_maps[c][in_names[i]])
                            for c in range(n_cores)], axis=0), sh)
        for i in range(n_params)
    ]
    concat_zeros = [
        jax.device_put(np.zeros((n_cores * z.shape[0], *z.shape[1:]), z.dtype), sh)
        for z in zero_outs
    ]
    return fn, concat_in, concat_zeros, out_names, out_avals


def _run_once(nc, in_maps, n_cores):
    import jax
    fn, concat_in, concat_zeros, out_names, out_avals = _make_fn(
        nc, in_maps, n_cores)
    out_arrs = fn(*concat_in, *concat_zeros)
    jax.block_until_ready(out_arrs)
    np_outs = [np.asarray(a) for a in out_arrs]
    return [
        {name: np_outs[i].reshape(n_cores, *out_avals[i].shape)[c]
         for i, name in enumerate(out_names)}
        for c in range(n_cores)
    ]


def _bench_slope(nc, in_maps, n_cores, m1=2, m2=12, reps=6):
    """Steady-state per-dispatch time: slope of wall time vs queued
    dispatches (cancels the fixed axon-tunnel latency)."""
    import time
    import jax
    fn, concat_in, concat_zeros, _, _ = _make_fn(nc, in_maps, n_cores)
    o = fn(*concat_in, *concat_zeros)
    jax.block_until_ready(o)

    def batch(m):
        t0 = time.perf_counter()
        outs = None
        for _ in range(m):
            outs = fn(*concat_in, *concat_zeros)
        jax.block_until_ready(outs)
        return time.perf_counter() - t0

    slopes = []
    for _ in range(reps):
        t1 = batch(m1)
        t2 = batch(m2)
        slopes.append((t2 - t1) / (m2 - m1))
    return int(min(slopes) * 1e9)


def kernel(nodes, edges, senders, receivers, Ws_k, Ws_b, Wr_k, Wr_b, We_k,
           We_b, a):
    global LAST_EXEC_NS, LAST_BENCH_NS

    nodes = np.asarray(nodes, dtype=np.float32)
    edges = np.asarray(edges, dtype=np.float32)
    senders = np.asarray(senders, dtype=np.int32)
    receivers = np.asarray(receivers, dtype=np.int32)
    Ws_k = np.asarray(Ws_k, dtype=np.float32)
    Wr_k = np.asarray(Wr_k, dtype=np.float32)
    We_k = np.asarray(We_k, dtype=np.float32)
    a = np.asarray(a, dtype=np.float32)
    assert not np.any(np.asarray(Ws_b)) and not np.any(np.asarray(Wr_b)) \
        and not np.any(np.asarray(We_b)), "nonzero biases not supported"

    d = _prep(nodes, edges, senders, receivers, NCORES)
    C, CW = _const_block(Ws_k, Wr_k, We_k, a)

    nc = _build(d["NTILES"], d["T_max"], d["NCHUNK"], d["E_pad"],
                d["NLOC_PAD"], CW)

    in_maps = [
        dict(EDG=d["EDG"][ci], SRT=d["SRT"][ci], RROW=d["RROW"][ci],
             RLOC=d["RLOC"][ci], NLT=d["NLT"][ci], CONST=C)
        for ci in range(NCORES)
    ]
    results = _run_once(nc, in_maps, NCORES)

    bench = int(os.environ.get("GAT_BENCH", "0"))
    if bench > 0:
        rep = int(os.environ.get("GAT_REPEAT", "16"))
        nc_r = _build(d["NTILES"], d["T_max"], d["NCHUNK"], d["E_pad"],
                      d["NLOC_PAD"], CW, repeat=rep)
        per_call = _bench_slope(nc_r, in_maps, NCORES)
        LAST_BENCH_NS = LAST_EXEC_NS = int(per_call / rep)

    allout = np.concatenate(
        [results[ci]["OUT"] for ci in range(NCORES)], axis=0
    )
    out = allout[d["pos_of_node"]]
    return out.astype(np.float32)
